# revision 1
# baseline (speedup 1.0000x reference)
"""GSAPool pairwise-distance + mean-threshold adjacency kernel for TRN2.

dist[b,i,j] = sqrt(||x_i||^2 + ||y_j||^2 - 2 x_i.y_j), mask = dist <= mean_b(dist)

Sharding: pure data-parallel over batch b: 64 samples -> 8 cores x 8 samples.
Per sample on a core:
  - load x,y [1024,256] natural layout
  - xx[m] row norms via DVE tensor_tensor_reduce (mult+add)
  - PE-transpose x,y to d-major; x side scaled by -2 on copy-out
  - yy[n] via ones-matmul over ysq (partition reduction on PE)
  - dist^2 psum = (-2x)^T y  (+ rank-1 ones x yy row), K=256 in 2 k-tiles
  - ACT: dist = sqrt(psum + xx bias), fused accum_out row sums for the mean
  - mean via ones-matmul + reduce + broadcast matmul
  - DVE tensor_scalar is_le -> u8 mask
Outputs: dist f32, mask u8 (cast to bool on host).
"""

import numpy as np
from contextlib import ExitStack

import concourse.bass as bass
import concourse.tile as tile
from concourse import bacc, mybir
from concourse.bass_utils import run_bass_kernel_spmd
from concourse.masks import make_identity

B = 64
M = 1024
N = 1024
D = 256
P = 128
MT = M // P        # 8 m-tiles
NCORES = 8
S = B // NCORES    # 8 samples per core
F32 = mybir.dt.float32
U8 = mybir.dt.uint8
ALU = mybir.AluOpType
ACTF = mybir.ActivationFunctionType


def build_body(ctx, tc, x_d, y_d, dist_d, mask_d, n_samples):
    nc = tc.nc

    const_pool = ctx.enter_context(tc.tile_pool(name="const", bufs=1))
    ident = const_pool.tile([P, P], F32)
    make_identity(nc, ident[:])
    ones_col = const_pool.tile([P, 8], F32)
    nc.gpsimd.memset(ones_col[:], 1.0)
    # [2, P] weights: row0 = ones, row1 = zeros — K=2 stand-in for rank-1
    # updates (K=1 matmuls are an unusual PE shape; avoid them).
    ones_row2 = const_pool.tile([2, P], F32)
    nc.gpsimd.memset(ones_row2[:, :], 0.0)
    nc.gpsimd.memset(ones_row2[0:1, :], 1.0)
    zeros_bias = const_pool.tile([P, 1], F32)
    nc.gpsimd.memset(zeros_bias[:], 0.0)

    nat_pool = ctx.enter_context(tc.tile_pool(name="nat", bufs=2))
    tr_pool = ctx.enter_context(tc.tile_pool(name="tr", bufs=2))
    dist_pool = ctx.enter_context(tc.tile_pool(name="dist", bufs=10))
    mask_pool = ctx.enter_context(tc.tile_pool(name="mask", bufs=2))
    small_pool = ctx.enter_context(tc.tile_pool(name="small", bufs=2))
    psum_tr = ctx.enter_context(tc.tile_pool(name="psum_tr", bufs=3, space="PSUM"))
    psum_d2 = ctx.enter_context(tc.tile_pool(name="psum_d2", bufs=3, space="PSUM"))
    psum_sm = ctx.enter_context(tc.tile_pool(name="psum_sm", bufs=2, space="PSUM"))

    for s in range(n_samples):
        # ---- loads (natural layout, m-tile t at free cols [t*D, (t+1)*D)) ----
        x_nat = nat_pool.tile([P, MT * D], F32, tag="x_nat")
        nc.sync.dma_start(
            out=x_nat.rearrange("p (t d) -> p t d", t=MT),
            in_=x_d[s].rearrange("(t p) d -> p t d", p=P),
        )
        y_nat = nat_pool.tile([P, MT * D], F32, tag="y_nat")
        nc.sync.dma_start(
            out=y_nat.rearrange("p (t d) -> p t d", t=MT),
            in_=y_d[s].rearrange("(t p) d -> p t d", p=P),
        )

        # ---- xx row norms: xx8[p, t] = sum_d x[128t+p, d]^2 ----
        # (ACT Square + fused accumulate; tensor_tensor_reduce faults the
        #  exec unit through this compile path, so keep it off.)
        xx8 = small_pool.tile([P, MT], F32, tag="xx8")
        for t in range(MT):
            sq_scratch = small_pool.tile([P, D], F32, tag="sq_scratch")
            nc.scalar.activation(
                sq_scratch[:],
                x_nat[:, t * D:(t + 1) * D],
                ACTF.Square,
                bias=zeros_bias[:, 0:1],
                scale=1.0,
                accum_out=xx8[:, t:t + 1],
            )

        # ---- PE transposes to d-major; x scaled by -2 on copy-out ----
        xTm2 = tr_pool.tile([P, 2 * M], F32, tag="xTm2")  # k-tile kt at cols [kt*M, (kt+1)*M)
        yT = tr_pool.tile([P, 2 * N], F32, tag="yT")
        ysq = tr_pool.tile([P, 2 * N], F32, tag="ysq")
        for kt in range(2):
            for t in range(MT):
                ptrx = psum_tr.tile([P, P], F32, tag="ptr")
                nc.tensor.transpose(
                    ptrx[:],
                    x_nat[:, t * D + kt * P: t * D + kt * P + P],
                    ident[:],
                )
                nc.vector.tensor_scalar_mul(
                    xTm2[:, kt * M + t * P: kt * M + (t + 1) * P], ptrx[:], -2.0
                )
        for kt in range(2):
            for t in range(MT):
                ptry = psum_tr.tile([P, P], F32, tag="ptr")
                nc.tensor.transpose(
                    ptry[:],
                    y_nat[:, t * D + kt * P: t * D + kt * P + P],
                    ident[:],
                )
                nc.vector.tensor_copy(
                    yT[:, kt * N + t * P: kt * N + (t + 1) * P], ptry[:]
                )
                nc.scalar.square(
                    ysq[:, kt * N + t * P: kt * N + (t + 1) * P], ptry[:]
                )

        # ---- yy row [2, N] via ones-matmul over ysq (row1 zeroed) ----
        yyrow = small_pool.tile([2, N], F32, tag="yyrow")
        nc.gpsimd.memset(yyrow[:, :], 0.0)
        for nh in range(2):
            pyy = psum_sm.tile([8, 512], F32, tag="sm")
            for kt in range(2):
                nc.tensor.matmul(
                    pyy[:],
                    ones_col[:],
                    ysq[:, kt * N + nh * 512: kt * N + nh * 512 + 512],
                    start=(kt == 0),
                    stop=(kt == 1),
                )
            nc.scalar.copy(yyrow[0:1, nh * 512:(nh + 1) * 512], pyy[0:1, :])

        # ---- main matmuls + fused sqrt/rowsum ----
        rs = small_pool.tile([P, 2 * MT], F32, tag="rs")
        dist_tiles = []
        for i in range(MT):
            dt_tile = dist_pool.tile([P, N], F32, tag="dist")
            for nh in range(2):
                pd2 = psum_d2.tile([P, 512], F32, tag="pd2")
                for kt in range(2):
                    nc.tensor.matmul(
                        pd2[:],
                        xTm2[:, kt * M + i * P: kt * M + (i + 1) * P],
                        yT[:, kt * N + nh * 512: kt * N + nh * 512 + 512],
                        start=(kt == 0),
                        stop=False,
                    )
                nc.tensor.matmul(
                    pd2[:],
                    ones_row2[:],
                    yyrow[:, nh * 512:(nh + 1) * 512],
                    start=False,
                    stop=True,
                )
                nc.scalar.activation(
                    dt_tile[:, nh * 512:(nh + 1) * 512],
                    pd2[:],
                    ACTF.Sqrt,
                    bias=xx8[:, i:i + 1],
                    scale=1.0,
                    accum_out=rs[:, 2 * i + nh: 2 * i + nh + 1],
                )
            nc.sync.dma_start(out=dist_d[s, i * P:(i + 1) * P, :], in_=dt_tile[:])
            dist_tiles.append(dt_tile)

        # ---- mean: total = sum(rs) over partitions and free ----
        ptot = psum_sm.tile([8, 2 * MT], F32, tag="sm")
        nc.tensor.matmul(ptot[:], ones_col[:], rs[:], start=True, stop=True)
        tot = small_pool.tile([2, 8], F32, tag="tot")
        nc.gpsimd.memset(tot[:, :], 0.0)
        nc.vector.tensor_reduce(
            out=tot[0:1, 0:1], in_=ptot[0:1, :], axis=mybir.AxisListType.X, op=ALU.add
        )
        pavg = psum_sm.tile([P, 8], F32, tag="sm")
        nc.tensor.matmul(pavg[:], ones_row2[:], tot[:], start=True, stop=True)
        avg = small_pool.tile([P, 1], F32, tag="avg")
        nc.scalar.activation(
            avg[:], pavg[:, 0:1], ACTF.Copy, bias=0.0, scale=1.0 / float(M * N)
        )

        # ---- compare + mask out ----
        mask_all = mask_pool.tile([P, MT * N], U8, tag="mask")
        for i in range(MT):
            nc.vector.tensor_scalar(
                mask_all[:, i * N:(i + 1) * N],
                dist_tiles[i][:],
                avg[:, 0:1],
                None,
                ALU.is_le,
            )
        nc.sync.dma_start(
            out=mask_d[s].rearrange("(t p) n -> p t n", p=P),
            in_=mask_all.rearrange("p (t n) -> p t n", t=MT),
        )


def build_program(n_samples=S, num_devices=NCORES):
    nc = bacc.Bacc(
        "TRN2", target_bir_lowering=False, debug=False, num_devices=num_devices
    )
    x_d = nc.dram_tensor("x", [n_samples, M, D], F32, kind="ExternalInput").ap()
    y_d = nc.dram_tensor("y", [n_samples, N, D], F32, kind="ExternalInput").ap()
    dist_d = nc.dram_tensor("dist", [n_samples, M, N], F32, kind="ExternalOutput").ap()
    mask_d = nc.dram_tensor("mask", [n_samples, M, N], U8, kind="ExternalOutput").ap()
    with tile.TileContext(nc) as tc:
        with ExitStack() as ctx:
            build_body(ctx, tc, x_d, y_d, dist_d, mask_d, n_samples)
    nc.compile()
    return nc


_nc_cache = None


def _get_nc():
    global _nc_cache
    if _nc_cache is None:
        _nc_cache = build_program()
    return _nc_cache


def kernel(x, y):
    x = np.ascontiguousarray(np.asarray(x), dtype=np.float32).reshape(B, M, D)
    y = np.ascontiguousarray(np.asarray(y), dtype=np.float32).reshape(B, N, D)
    nc = _get_nc()
    in_maps = [
        {
            "x": np.ascontiguousarray(x[c * S:(c + 1) * S]),
            "y": np.ascontiguousarray(y[c * S:(c + 1) * S]),
        }
        for c in range(NCORES)
    ]
    res = run_bass_kernel_spmd(nc, in_maps, list(range(NCORES)))
    dist = np.concatenate([res.results[c]["dist"] for c in range(NCORES)], axis=0)
    mask = np.concatenate([res.results[c]["mask"] for c in range(NCORES)], axis=0)
    return dist, mask != 0



# revision 8
# speedup vs baseline: 334.9048x; 334.9048x over previous
"""GSAPool pairwise-distance + mean-threshold adjacency kernel for TRN2 (v5).

dist[b,i,j] = sqrt(||x_i||^2 + ||y_j||^2 - 2 x_i.y_j), mask = dist <= mean_b(dist)
Sharding: pure data-parallel over batch b: 64 samples -> 8 cores x 8 samples.

Host preps layout-only input derivatives (O(n*d), 0.2% of FLOPs):
xt=(-2x)^T f16, yt=y^T f16, xx row norms f32, yy row norms as f16 hi/lo pair
(hi+lo reconstructs f32 exactly to ~1e-4).

Device per sample:
  - fp16 matmuls (1 cyc/row vs 4 for fp32): psum = (-2x)^T y + ones x yy
    (K=2 fp16 rank-1 completes + yy[j] in full precision via hi/lo rows)
  - DVE evac: dist2 = psum + xx[P,1] (TensorScalarPtr, f32, SBUF)
  - ACT sqrt -> dist fp16 output (halves the dominant HBM write) with
    fused accum_out row sums (mean of true dist values)
  - mean via two tiny PE ones-matmuls + DVE reduce (all fast-path HW ops;
    gpsimd ucode reductions/broadcasts measured ~100us/core slower - avoided)
  - mask compare in dist^2 domain: dist2 <= avg^2 f32 on DVE (fp16 output
    rounding never touches the mask). Mean+mask pipelined one sample behind.
"""

import numpy as np
from concurrent.futures import ThreadPoolExecutor
from contextlib import ExitStack

import concourse.bass as bass
import concourse.tile as tile
from concourse import bacc, mybir
from concourse.bass_utils import run_bass_kernel_spmd

B = 64
M = 1024
N = 1024
D = 256
P = 128
MT = M // P        # 8 m-tiles
KT = D // P        # 2 k-tiles
NCORES = 8
S = B // NCORES    # 8 samples per core
F32 = mybir.dt.float32
F16 = mybir.dt.float16
U8 = mybir.dt.uint8
ALU = mybir.AluOpType
ACTF = mybir.ActivationFunctionType


def build_body(ctx, tc, xt_d, yt_d, xx_d, yy2_d, dist_d, mask_d, n_samples):
    nc = tc.nc

    const_pool = ctx.enter_context(tc.tile_pool(name="const", bufs=1))
    ones2_16 = const_pool.tile([2, P], F16)
    nc.gpsimd.memset(ones2_16[:, :], 1.0)
    ones_col = const_pool.tile([P, 8], F32)
    nc.gpsimd.memset(ones_col[:], 1.0)
    ones_row2 = const_pool.tile([2, P], F32)
    nc.gpsimd.memset(ones_row2[:, :], 0.0)
    nc.gpsimd.memset(ones_row2[0:1, :], 1.0)
    zeros_bias = const_pool.tile([P, 1], F32)
    nc.gpsimd.memset(zeros_bias[:], 0.0)
    # tot is reused across samples: rows 1..7 / cols 1..7 stay zero, only
    # [0,0] is rewritten by each sample's reduce (avoids per-sample Q7 memset)
    tot = const_pool.tile([2, 8], F32)
    nc.gpsimd.memset(tot[:, :], 0.0)

    in_pool = ctx.enter_context(tc.tile_pool(name="inp", bufs=2))
    d2_pool = ctx.enter_context(tc.tile_pool(name="d2", bufs=3))
    d16_pool = ctx.enter_context(tc.tile_pool(name="d16", bufs=2))
    mask_pool = ctx.enter_context(tc.tile_pool(name="mask", bufs=2))
    small_pool = ctx.enter_context(tc.tile_pool(name="small", bufs=2))
    yy_pool = ctx.enter_context(tc.tile_pool(name="yy", bufs=2))
    psum_d2 = ctx.enter_context(tc.tile_pool(name="psum_d2", bufs=3, space="PSUM"))
    psum_sm = ctx.enter_context(tc.tile_pool(name="psum_sm", bufs=2, space="PSUM"))

    def emit_mask(s, dist2, rs, last):
        # mean -> avg^2 [P,1] f32 via tiny PE matmuls (fast path)
        ptot = psum_sm.tile([8, MT], F32, tag="sm")
        nc.tensor.matmul(ptot[:], ones_col[:], rs[:], start=True, stop=True)
        nc.vector.tensor_reduce(
            out=tot[0:1, 0:1], in_=ptot[0:1, :], axis=mybir.AxisListType.X, op=ALU.add
        )
        pavg = psum_sm.tile([P, 8], F32, tag="sm")
        nc.tensor.matmul(pavg[:], ones_row2[:], tot[:], start=True, stop=True)
        avg2 = small_pool.tile([P, 1], F32, tag="avg2")
        nc.scalar.activation(
            avg2[:], pavg[:, 0:1], ACTF.Square, bias=zeros_bias[:, 0:1],
            scale=1.0 / float(M * N),
        )
        # mask: dist2 <= avg^2, u8, DVE
        mask_all = mask_pool.tile([P, MT * N], U8, tag="mask")
        for i in range(MT):
            nc.vector.tensor_scalar(
                mask_all[:, i * N:(i + 1) * N],
                dist2[:, i * N:(i + 1) * N],
                avg2[:, 0:1],
                None,
                ALU.is_le,
            )
        nchunk = 4 if last else 2
        for h in range(nchunk):
            hw = MT // nchunk * N
            nc.sync.dma_start(
                out=mask_d[s, h * (M // nchunk):(h + 1) * (M // nchunk)].rearrange(
                    "(t p) n -> p t n", p=P),
                in_=mask_all[:, h * hw:(h + 1) * hw].rearrange(
                    "p (t n) -> p t n", t=MT // nchunk),
            )

    prev = None
    for s in range(n_samples):
        # ---- loads (small first) ----
        xx8 = small_pool.tile([P, MT], F32, tag="xx8")
        nc.sync.dma_start(out=xx8[:], in_=xx_d[s])
        yy2 = yy_pool.tile([2, N], F16, tag="yy2")
        nc.sync.dma_start(out=yy2[:], in_=yy2_d[s])
        xtile = in_pool.tile([P, KT * M], F16, tag="xt")  # col = kt*M+m, holds -2x^T
        ytile = in_pool.tile([P, KT * N], F16, tag="yt")  # col = kt*N + j
        for kt in range(KT):
            nc.sync.dma_start(out=xtile[:, kt * M:(kt + 1) * M], in_=xt_d[s, kt])
            nc.sync.dma_start(out=ytile[:, kt * N:(kt + 1) * N], in_=yt_d[s, kt])

        # ---- per m-tile: fp16 matmuls (+ yy rank-1) -> DVE evac -> sqrt ----
        dist2 = d2_pool.tile([P, MT * N], F32, tag="dist2")
        dist16 = d16_pool.tile([P, MT * N], F16, tag="dist16")
        rs = small_pool.tile([P, MT], F32, tag="rs")
        for i in range(MT):
            pd2 = psum_d2.tile([P, N], F32, tag="pd2")
            for nh in range(2):
                for kt in range(KT):
                    nc.tensor.matmul(
                        pd2[:, nh * 512:(nh + 1) * 512],
                        xtile[:, kt * M + i * P: kt * M + (i + 1) * P],
                        ytile[:, kt * N + nh * 512: kt * N + nh * 512 + 512],
                        start=(kt == 0),
                        stop=False,
                    )
                nc.tensor.matmul(
                    pd2[:, nh * 512:(nh + 1) * 512],
                    ones2_16[:],
                    yy2[:, nh * 512:(nh + 1) * 512],
                    start=False,
                    stop=True,
                )
            c0 = i * N
            nc.vector.tensor_scalar(
                dist2[:, c0:c0 + N], pd2[:], xx8[:, i:i + 1], None, ALU.add,
            )
            nc.scalar.activation(
                dist16[:, c0:c0 + N],
                dist2[:, c0:c0 + N],
                ACTF.Sqrt,
                bias=zeros_bias[:, 0:1],
                scale=1.0,
                accum_out=rs[:, i:i + 1],
            )
            if i == MT // 2 - 1 or i == MT - 1:
                h = 0 if i < MT // 2 else 1
                hw = MT // 2 * N
                nc.sync.dma_start(
                    out=dist_d[s, h * (M // 2):(h + 1) * (M // 2)].rearrange(
                        "(t p) n -> p t n", p=P),
                    in_=dist16[:, h * hw:(h + 1) * hw].rearrange(
                        "p (t n) -> p t n", t=MT // 2),
                )

        # ---- mean chain + mask pipelined one sample behind ----
        if prev is not None:
            emit_mask(*prev)
        prev = (s, dist2, rs, s == n_samples - 1)
    emit_mask(*prev)


def build_program(n_samples=S, num_devices=NCORES, repeats=1):
    nc = bacc.Bacc(
        "TRN2", target_bir_lowering=False, debug=False, num_devices=num_devices
    )
    xt_d = nc.dram_tensor("xt", [n_samples, KT, P, M], F16, kind="ExternalInput").ap()
    yt_d = nc.dram_tensor("yt", [n_samples, KT, P, N], F16, kind="ExternalInput").ap()
    xx_d = nc.dram_tensor("xx", [n_samples, P, MT], F32, kind="ExternalInput").ap()
    yy2_d = nc.dram_tensor("yy2", [n_samples, 2, N], F16, kind="ExternalInput").ap()
    dist_d = nc.dram_tensor("dist", [n_samples, M, N], F16, kind="ExternalOutput").ap()
    mask_d = nc.dram_tensor("mask", [n_samples, M, N], U8, kind="ExternalOutput").ap()
    with tile.TileContext(nc) as tc:
        for _ in range(repeats):
            with ExitStack() as ctx:
                build_body(ctx, tc, xt_d, yt_d, xx_d, yy2_d, dist_d, mask_d,
                           n_samples)
    nc.compile()
    return nc


_nc_cache = None


def _get_nc():
    global _nc_cache
    if _nc_cache is None:
        _nc_cache = build_program()
    return _nc_cache


def _prep_core(x_c, y_c):
    """Host layout prep for one core's samples: O(n*d) only."""
    xm2 = x_c * np.float32(-2.0)
    xt = xm2.transpose(0, 2, 1).astype(np.float16).reshape(S, KT, P, M)
    yt = y_c.transpose(0, 2, 1).astype(np.float16).reshape(S, KT, P, N)
    xx = np.einsum("smd,smd->sm", x_c, x_c).reshape(S, MT, P).transpose(0, 2, 1)
    yyr = np.einsum("snd,snd->sn", y_c, y_c)
    yy_hi = yyr.astype(np.float16)
    yy_lo = (yyr - yy_hi.astype(np.float32)).astype(np.float16)
    yy2 = np.stack([yy_hi, yy_lo], axis=1)
    return {
        "xt": np.ascontiguousarray(xt),
        "yt": np.ascontiguousarray(yt),
        "xx": np.ascontiguousarray(xx),
        "yy2": np.ascontiguousarray(yy2),
    }


def kernel(x, y):
    x = np.ascontiguousarray(np.asarray(x), dtype=np.float32).reshape(B, M, D)
    y = np.ascontiguousarray(np.asarray(y), dtype=np.float32).reshape(B, N, D)
    nc = _get_nc()
    with ThreadPoolExecutor(max_workers=8) as ex:
        in_maps = list(ex.map(
            lambda c: _prep_core(x[c * S:(c + 1) * S], y[c * S:(c + 1) * S]),
            range(NCORES),
        ))
    res = run_bass_kernel_spmd(nc, in_maps, list(range(NCORES)))
    with ThreadPoolExecutor(max_workers=8) as ex:
        dists = list(ex.map(
            lambda c: res.results[c]["dist"].astype(np.float32), range(NCORES)
        ))
    dist = np.concatenate(dists, axis=0)
    mask = np.concatenate([res.results[c]["mask"] for c in range(NCORES)], axis=0)
    return dist, mask != 0
